# revision 9
# baseline (speedup 1.0000x reference)
"""Conformer block (MHSA w/ rel-pos shift + ConvModule + FFN) on 8 TRN2 cores.

Sharding: data-parallel over batch (2 batch elements per core); the only
cross-core communication is a [128, 2] AllReduce for the ConvModule's
BatchNorm statistics.

Layout: activations live transposed in SBUF as [feature(partitions), s(free)].
All matmuls run in bf16 (fp32 PSUM accumulate); score rel-shift runs through a
padded flat DRAM buffer re-read with an overlapping strided AP, and the
[s,t]->[t,s] orientation flip rides the same read via DMA-transpose.
exp(content + shifted) is computed as exp(content) * exp(shifted) so each
factor's exp rides a mandatory PSUM-evacuation pass on the scalar engine.
"""
import math
import numpy as np
import ml_dtypes

import concourse.bass as bass
import concourse.bacc as bacc
import concourse.tile as tile
import concourse.mybir as mybir
from concourse.bass_utils import run_bass_kernel_spmd
from concourse.ap import AP

B, S, D, H, KS = 16, 1024, 128, 4, 31
DH = D // H            # 32
EXP = 4
C2 = 2 * D             # 256
F = EXP * D            # 512
NCORES = 8
BPC = B // NCORES      # 2 batch elements per core
PAD = KS // 2          # 15
SP = S + 2 * PAD       # 1054
EPS = 1e-5
ISQ = 1.0 / math.sqrt(float(D))
NB = S * (S + 1)       # flat A' buffer length (rows of S+1 incl. zero col)

f32 = mybir.dt.float32
f32r = mybir.dt.float32r
bf16 = mybir.dt.bfloat16
FT = mybir.ActivationFunctionType
ALU = mybir.AluOpType

TRACE = False
LAST_EXEC_TIME_NS = None
LAST_TRACE_PATH = None
_CACHE = {}

# vecs bundle column map (all [128] fp32 vectors, one per column)
VC = ["ln_attn_w", "ln_attn_b", "qu_bias", "qv_bias", "bk", "dcnn_b",
      "cm_ln_w", "cm_ln_b", "bn_w", "bn_b", "ff_ln_w", "ff_ln_b",
      "pw1_b0", "pw1_b1", "ff_b1_0", "ff_b1_1", "ff_b1_2", "ff_b1_3"]
VCI = {n: i for i, n in enumerate(VC)}

# rowvecs bundle (bf16 [1, 6*128]): bias rows used by K=1 psum-init matmuls
RV = ["bv", "bo", "ff_b2", "dw_b0", "dw_b1", "pw2_b"]
RVI = {n: i for i, n in enumerate(RV)}


def _pe_host():
    pos = np.arange(S, dtype=np.float64)[:, None]
    div = np.exp(np.arange(0, D, 2, dtype=np.float64) * (-math.log(10000.0) / D))
    pe = np.zeros((S, D))
    pe[:, 0::2] = np.sin(pos * div)
    pe[:, 1::2] = np.cos(pos * div)
    return pe


def _build():
    nc = bacc.Bacc()

    # ---- DRAM parameters (per-core shards, marshaled on host) ----
    xT = nc.declare_dram_parameter("xT", [BPC, D, S], f32, isOutput=False)
    vecs = nc.declare_dram_parameter("vecs", [D, len(VC)], f32, isOutput=False)
    rowv = nc.declare_dram_parameter("rowv", [1, len(RV) * D], bf16,
                                     isOutput=False)
    wqT = nc.declare_dram_parameter("wqT", [D, D], bf16, isOutput=False)
    wkT = nc.declare_dram_parameter("wkT", [D, D], bf16, isOutput=False)
    wvT = nc.declare_dram_parameter("wvT", [D, D], bf16, isOutput=False)
    wpT = nc.declare_dram_parameter("wpT", [D, D], bf16, isOutput=False)
    woT = nc.declare_dram_parameter("woT", [D, D], bf16, isOutput=False)
    peT = nc.declare_dram_parameter("peT", [D, S], bf16, isOutput=False)
    dcnnT = nc.declare_dram_parameter("dcnnT", [D, KS, D], bf16, isOutput=False)
    pw1T = nc.declare_dram_parameter("pw1T", [D, C2], bf16, isOutput=False)
    dwT = nc.declare_dram_parameter("dwT", [D, 2, KS, 2, D], bf16,
                                    isOutput=False)
    pw2T = nc.declare_dram_parameter("pw2T", [D, D], bf16, isOutput=False)
    ff1T = nc.declare_dram_parameter("ff1T", [D, F], bf16, isOutput=False)
    ff2T = nc.declare_dram_parameter("ff2T", [D, EXP, D], bf16, isOutput=False)

    outT = nc.declare_dram_parameter("outT", [BPC, D, S], f32, isOutput=True)

    with tile.TileContext(nc) as tc:
        with tc.tile_pool(name="wp", bufs=1) as wp, \
             tc.tile_pool(name="bp", bufs=2) as bp, \
             tc.tile_pool(name="tp", bufs=1) as tp, \
             tc.tile_pool(name="pk", bufs=5, space="PSUM") as pk, \
             tc.tile_pool(name="pz", bufs=3, space="PSUM") as pz, \
             tc.tile_pool(name="dr", bufs=1, space="DRAM") as dr:

            # ================= constants & weights =================
            s_vec = wp.tile([D, len(VC)], f32, name="s_vec")
            nc.gpsimd.dma_start(out=s_vec, in_=vecs[:])
            s_rowv = wp.tile([1, len(RV) * D], bf16, name="s_rowv")
            nc.gpsimd.dma_start(out=s_rowv, in_=rowv[:])

            def vcol(nm):
                return s_vec[:, VCI[nm]:VCI[nm] + 1]

            def rrow(nm):
                return s_rowv[:, RVI[nm] * D:(RVI[nm] + 1) * D]

            s_wqT = wp.tile([D, D], bf16, name="s_wqT")
            nc.gpsimd.dma_start(out=s_wqT, in_=wqT[:])
            s_wkT = wp.tile([D, D], bf16, name="s_wkT")
            nc.gpsimd.dma_start(out=s_wkT, in_=wkT[:])
            s_wvT = wp.tile([D, D], bf16, name="s_wvT")
            nc.gpsimd.dma_start(out=s_wvT, in_=wvT[:])
            s_wpT = wp.tile([D, D], bf16, name="s_wpT")
            nc.gpsimd.dma_start(out=s_wpT, in_=wpT[:])
            s_woT = wp.tile([D, D], bf16, name="s_woT")
            nc.gpsimd.dma_start(out=s_woT, in_=woT[:])
            s_peT = wp.tile([D, S], bf16, name="s_peT")
            nc.gpsimd.dma_start(out=s_peT, in_=peT[:])
            s_dcnnT = wp.tile([D, KS, D], bf16, name="s_dcnnT")
            nc.gpsimd.dma_start(out=s_dcnnT, in_=dcnnT[:])
            s_pw1T = wp.tile([D, C2], bf16, name="s_pw1T")
            nc.gpsimd.dma_start(out=s_pw1T, in_=pw1T[:])
            s_dwT = wp.tile([D, 2, KS, 2, D], bf16, name="s_dwT")
            nc.gpsimd.dma_start(out=s_dwT, in_=dwT[:])
            s_pw2T = wp.tile([D, D], bf16, name="s_pw2T")
            nc.gpsimd.dma_start(out=s_pw2T, in_=pw2T[:])
            s_ff1T = wp.tile([D, F], bf16, name="s_ff1T")
            nc.gpsimd.dma_start(out=s_ff1T, in_=ff1T[:])
            s_ff2T = wp.tile([D, EXP, D], bf16, name="s_ff2T")
            nc.gpsimd.dma_start(out=s_ff2T, in_=ff2T[:])

            ones_col = wp.tile([D, 1], f32, name="ones_col")
            nc.vector.memset(ones_col, 1.0)
            ones_row_b = wp.tile([1, 512], bf16, name="ones_row_b")
            nc.vector.memset(ones_row_b, 1.0)
            ones_row_f = wp.tile([1, 512], f32, name="ones_row_f")
            nc.vector.memset(ones_row_f, 1.0)
            ones_row_r = wp.tile([1, 512], f32r, name="ones_row_r")
            nc.vector.tensor_copy(out=ones_row_r, in_=ones_row_f)
            ones_col_r = wp.tile([D, 1], f32r, name="ones_col_r")
            nc.vector.tensor_copy(out=ones_col_r, in_=ones_col)
            ones32 = wp.tile([D, DH], bf16, name="ones32")
            nc.vector.memset(ones32, 1.0)
            eps_col = wp.tile([D, 1], f32, name="eps_col")
            nc.vector.memset(eps_col, EPS)
            eps1 = wp.tile([1, 1], f32, name="eps1")
            nc.vector.memset(eps1, EPS)

            # p_aug: [128, 1025] bf16, col 0 = zeros, col 1+t = (wp @ pe.T)[:, t]
            paug = wp.tile([D, S + 1], bf16, name="paug")
            nc.vector.memset(paug[:, 0:1], 0.0)
            for c in range(2):
                pp = pk.tile([D, 512], f32, tag="pk")
                nc.tensor.matmul(pp, s_wpT, s_peT[:, c * 512:(c + 1) * 512],
                                 start=True, stop=True)
                nc.scalar.copy(out=paug[:, 1 + c * 512:1 + (c + 1) * 512],
                               in_=pp)

            # ================= helpers =================
            def part_ln_rows(src, name):
                """Partition-dim LN stats of src [128, 1024] f32.
                Returns (grow, hrow) f32r rows [1, 1024]: grow = rstd,
                hrow = mean * rstd."""
                sqt = tp.tile([D, S], f32r, name=f"sq_{name}", tag="f4k",
                              bufs=4, padded_shape=None)
                nc.vector.tensor_mul(out=sqt, in0=src, in1=src)
                rows = tp.tile([1, 2, S], f32, name=f"rows_{name}", tag="rows",
                               bufs=1)
                for c in range(2):
                    p1 = pk.tile([1, 512], f32, tag="pk")
                    nc.tensor.matmul(p1, ones_col,
                                     src[:, c * 512:(c + 1) * 512],
                                     start=True, stop=True)
                    nc.vector.tensor_scalar_mul(
                        out=rows[:, 0, c * 512:(c + 1) * 512], in0=p1,
                        scalar1=1.0 / D)
                    p2 = pk.tile([1, 512], f32, tag="pk")
                    nc.tensor.matmul(p2, ones_col_r,
                                     sqt[:, c * 512:(c + 1) * 512],
                                     start=True, stop=True)
                    nc.vector.tensor_scalar_mul(
                        out=rows[:, 1, c * 512:(c + 1) * 512], in0=p2,
                        scalar1=1.0 / D)
                m = rows[:, 0, :]
                msq = rows[:, 1, :]
                var = tp.tile([1, S], f32, name=f"var_{name}", tag="var",
                              bufs=1)
                nc.vector.tensor_mul(out=var, in0=m, in1=m)
                nc.vector.tensor_tensor(out=var, in0=msq, in1=var,
                                        op=ALU.subtract)
                grow = tp.tile([1, S], f32r, name=f"grow_{name}", tag="grow",
                               bufs=1)
                hrow = tp.tile([1, S], bf16, name=f"hrow_{name}", tag="hrow",
                               bufs=1)
                # rstd = exp(-0.5 * ln(var + eps))  (stays in exp/ln ACT set)
                nc.scalar.activation(out=var, in_=var, func=FT.Ln, bias=eps1,
                                     scale=1.0)
                nc.scalar.activation(out=grow, in_=var, func=FT.Exp,
                                     scale=-0.5)
                nc.vector.tensor_mul(out=hrow, in0=m, in1=grow)
                return grow, hrow

            def apply_rows(src, grow, hrow, wname, bname, out_tile):
                """out = (src * grow - hrow) * w + b; grow/hrow [1,*] rows
                broadcast across partitions via K=1 matmuls."""
                tmp = tp.tile([D, S], f32, name="ln_tmp", tag="f4k", bufs=4)
                for c in range(2):
                    sl = slice(c * 512, (c + 1) * 512)
                    gb = pk.tile([D, 512], f32, tag="pk")
                    nc.tensor.matmul(gb, ones_row_r[:, 0:D], grow[:, sl],
                                     start=True, stop=True)
                    hb = pk.tile([D, 512], f32, tag="pk")
                    nc.tensor.matmul(hb, ones_row_b[:, 0:D], hrow[:, sl],
                                     start=True, stop=True)
                    nc.vector.tensor_mul(out=tmp[:, sl], in0=src[:, sl],
                                         in1=gb)
                    nc.vector.tensor_tensor(out=tmp[:, sl], in0=tmp[:, sl],
                                            in1=hb, op=ALU.subtract)
                nc.vector.tensor_scalar(out=out_tile, in0=tmp,
                                        scalar1=vcol(wname),
                                        scalar2=vcol(bname),
                                        op0=ALU.mult, op1=ALU.add)

            # ================= per-batch stage A =================
            x1p_f = []
            zglu = []
            bnmv = []
            for b in range(BPC):
                inT = bp.tile([D, S], f32, name=f"inT{b}", tag="inT")
                nc.gpsimd.dma_start(out=inT, in_=xT[b])

                # ---- LN(attn) -> xbf bf16 ----
                grow, hrow = part_ln_rows(inT, f"ln1_{b}")
                xbf = bp.tile([D, S], bf16, name=f"xbf{b}", tag="xbf")
                apply_rows(inT, grow, hrow, "ln_attn_w", "ln_attn_b", xbf)

                # ---- q, k, v ----
                qu = bp.tile([D, S], bf16, name=f"qu{b}", tag="qu")
                qv = bp.tile([D, S], bf16, name=f"qv{b}", tag="qv")
                kbf = bp.tile([D, S], bf16, name=f"kbf{b}", tag="kbf")
                for c in range(2):
                    sl = slice(c * 512, (c + 1) * 512)
                    pq = pk.tile([D, 512], f32, tag="pk")
                    nc.tensor.matmul(pq, s_wqT, xbf[:, sl], start=True,
                                     stop=True)
                    nc.scalar.activation(out=qu[:, sl], in_=pq,
                                         func=FT.Identity,
                                         bias=vcol("qu_bias"), scale=ISQ)
                    nc.scalar.activation(out=qv[:, sl], in_=pq,
                                         func=FT.Identity,
                                         bias=vcol("qv_bias"), scale=ISQ)
                    pkk = pk.tile([D, 512], f32, tag="pk")
                    nc.tensor.matmul(pkk, s_wkT, xbf[:, sl], start=True,
                                     stop=True)
                    nc.scalar.activation(out=kbf[:, sl], in_=pkk,
                                         func=FT.Identity, bias=vcol("bk"),
                                         scale=1.0)
                # v8[p, tc, h, 0:32] = v[t = tc*128 + p, 32h:32h+32]; col 32 = 1
                v8 = bp.tile([D, 8, H, DH + 1], bf16, name=f"v8{b}", tag="v8")
                nc.vector.memset(v8[:, :, :, DH:DH + 1], 1.0)
                for t8 in range(8):
                    pv = pk.tile([D, D], f32, tag="pk")
                    nc.tensor.matmul(pv, ones_row_b[:, 0:D], rrow("bv"),
                                     start=True, stop=False)
                    nc.tensor.matmul(pv, xbf[:, t8 * D:(t8 + 1) * D], s_wvT,
                                     start=False, stop=True)
                    nc.scalar.copy(
                        out=v8[:, t8, :, 0:DH],
                        in_=pv.rearrange("p (h d) -> p h d", h=H))

                # ---- A' (diag pos scores), exp'd, to DRAM ----
                dramAs = []
                for h in range(H):
                    ea8 = tp.tile([D, 8, S + 1], bf16, name="ea8", tag="ea8",
                                  bufs=1)
                    hsl = slice(DH * h, DH * (h + 1))
                    tpos = (DH * h, 0)
                    for mt in range(8):
                        msl = slice(mt * D, (mt + 1) * D)
                        for jc, (j0, j1) in enumerate([(0, 512), (512, 1024),
                                                       (1024, 1025)]):
                            pa = pk.tile([D, 512], f32, tag="pk")
                            nc.tensor.matmul(pa[:, 0:j1 - j0], qv[hsl, msl],
                                             paug[hsl, j0:j1],
                                             start=True, stop=True,
                                             tile_position=tpos)
                            nc.scalar.activation(
                                out=ea8[:, mt, j0:j1], in_=pa[:, 0:j1 - j0],
                                func=FT.Exp, scale=1.0)
                    dA = dr.tile([NB], bf16, name=f"dA{b}_{h}",
                                 tag=f"dA{b}_{h}")
                    dst = AP(dA[:].tensor, dA[:].offset,
                             [[S + 1, D], [(S + 1) * D, 8], [1, S + 1]])
                    nc.gpsimd.dma_start(out=dst, in_=ea8)
                    dramAs.append(dA)

                # ---- content scores + shifted product + ctx, per s-block ----
                ctxn = []
                for sb in range(2):
                    ssl = slice(sb * 512, (sb + 1) * 512)
                    pctx = pz.tile([D, 512], f32, tag="pz")
                    pzz = pz.tile([D, 512], f32, tag="pz")
                    ets = {}
                    for pair in ((0, 1), (2, 3)):
                        for h in pair:
                            ets[h] = tp.tile([D, 8, 512], bf16,
                                             name=f"et{h}", tag="et", bufs=2)
                        for tc in range(8):
                            for h in pair:
                                hsl = slice(DH * h, DH * (h + 1))
                                pc = pk.tile([D, 512], f32, tag="pk")
                                nc.tensor.matmul(
                                    pc, kbf[hsl, tc * D:(tc + 1) * D],
                                    qu[hsl, ssl], start=True, stop=True,
                                    tile_position=(DH * h, 0))
                                nc.scalar.activation(out=ets[h][:, tc, :],
                                                     in_=pc, func=FT.Exp,
                                                     scale=1.0)
                        for h in pair:
                            sh = tp.tile([D, 8, 512], bf16, name=f"sh{h}",
                                         tag="sh", bufs=2)
                            for mb in range(4):
                                m0 = sb * 512 + mb * 128
                                src = AP(dramAs[h][:].tensor,
                                         dramAs[h][:].offset + (m0 + 1) * S,
                                         [[S, D], [1, S]])
                                nc.sync.dma_start(
                                    out=sh[:, :, mb * 128:(mb + 1) * 128],
                                    in_=src, transpose=True)
                            nc.vector.tensor_mul(out=ets[h], in0=ets[h],
                                                 in1=sh)
                            for tc in range(8):
                                nc.tensor.matmul(
                                    pctx[DH * h:DH * (h + 1), :],
                                    v8[:, tc, h, :][:, 0:DH],
                                    ets[h][:, tc, :],
                                    start=(tc == 0), stop=(tc == 7),
                                    tile_position=(0, DH * h))
                                nc.tensor.matmul(
                                    pzz[DH * h:DH * (h + 1), :],
                                    ones32, ets[h][:, tc, :],
                                    start=(tc == 0), stop=(tc == 7),
                                    tile_position=(0, DH * h))
                    rbc = tp.tile([D, 512], f32, name="rbc", tag="rbc",
                                  bufs=1)
                    nc.vector.reciprocal(out=rbc, in_=pzz)
                    cu = tp.tile([D, 512], bf16, name="cu", tag="cu", bufs=1)
                    nc.scalar.copy(out=cu, in_=pctx)
                    cn = tp.tile([D, 512], bf16, name="cn", tag="cn", bufs=2)
                    nc.vector.tensor_mul(out=cn, in0=cu, in1=rbc)
                    ctxn.append(cn)

                # ---- out-proj + residual -> x1 (padded f32 + bf16) ----
                x1f = bp.tile([D, SP], f32, name=f"x1f{b}", tag="x1f")
                x1b = bp.tile([D, SP], bf16, name=f"x1b{b}", tag="x1b")
                nc.vector.memset(x1f[:, 0:PAD], 0.0)
                nc.vector.memset(x1f[:, S + PAD:], 0.0)
                nc.vector.memset(x1b[:, 0:PAD], 0.0)
                nc.vector.memset(x1b[:, S + PAD:], 0.0)
                for sb in range(2):
                    ssl = slice(sb * 512, (sb + 1) * 512)
                    px = pk.tile([D, 512], f32, tag="pk")
                    nc.tensor.matmul(px, rrow("bo"), ones_row_b,
                                     start=True, stop=False)
                    nc.tensor.matmul(px, s_woT, ctxn[sb], start=False,
                                     stop=True)
                    psl = slice(PAD + sb * 512, PAD + (sb + 1) * 512)
                    nc.vector.tensor_add(out=x1f[:, psl], in0=px,
                                         in1=inT[:, ssl])
                    nc.vector.tensor_copy(out=x1b[:, psl], in_=x1f[:, psl])

                # ---- dcnn conv + fused joint-LN/cm-LN ----
                y = tp.tile([D, S], f32, name=f"y{b}", tag="f4k", bufs=4)
                for sb in range(2):
                    py = pk.tile([D, 512], f32, tag="pk")
                    for k in range(KS):
                        nc.tensor.matmul(py, s_dcnnT[:, k, :],
                                         x1b[:, sb * 512 + k:
                                             sb * 512 + k + 512],
                                         start=(k == 0), stop=(k == KS - 1))
                    nc.scalar.activation(out=y[:, sb * 512:(sb + 1) * 512],
                                         in_=py, func=FT.Identity,
                                         bias=vcol("dcnn_b"), scale=1.0)

                # stats: per-s column sums + joint scalars, fused double LN
                sqt = tp.tile([D, S], f32r, name="sqy", tag="f4k", bufs=4)
                nc.vector.tensor_mul(out=sqt, in0=y, in1=y)
                rows = tp.tile([1, 2, S], f32, name="rowsy", tag="rows",
                               bufs=1)
                for c in range(2):
                    p1 = pk.tile([1, 512], f32, tag="pk")
                    nc.tensor.matmul(p1, ones_col,
                                     y[:, c * 512:(c + 1) * 512],
                                     start=True, stop=True)
                    nc.vector.tensor_scalar_mul(
                        out=rows[:, 0, c * 512:(c + 1) * 512], in0=p1,
                        scalar1=1.0 / D)
                    p2 = pk.tile([1, 512], f32, tag="pk")
                    nc.tensor.matmul(p2, ones_col_r,
                                     sqt[:, c * 512:(c + 1) * 512],
                                     start=True, stop=True)
                    nc.vector.tensor_scalar_mul(
                        out=rows[:, 1, c * 512:(c + 1) * 512], in0=p2,
                        scalar1=1.0 / D)
                mc = rows[:, 0, :]
                msq = rows[:, 1, :]
                varc = tp.tile([1, S], f32, name="varc", tag="var", bufs=1)
                nc.vector.tensor_mul(out=varc, in0=mc, in1=mc)
                nc.vector.tensor_tensor(out=varc, in0=msq, in1=varc,
                                        op=ALU.subtract)
                # joint scalars
                jrow = tp.tile([1, 4], f32, name="jrow", tag="jrow", bufs=1)
                nc.vector.tensor_reduce(out=jrow[:, 0:1], in_=mc,
                                        axis=mybir.AxisListType.X, op=ALU.add)
                nc.vector.tensor_reduce(out=jrow[:, 1:2], in_=msq,
                                        axis=mybir.AxisListType.X, op=ALU.add)
                nc.vector.tensor_scalar_mul(out=jrow[:, 0:1],
                                            in0=jrow[:, 0:1], scalar1=1.0 / S)
                nc.vector.tensor_scalar_mul(out=jrow[:, 1:2],
                                            in0=jrow[:, 1:2], scalar1=1.0 / S)
                nc.vector.tensor_mul(out=jrow[:, 2:3], in0=jrow[:, 0:1],
                                     in1=jrow[:, 0:1])
                nc.vector.tensor_tensor(out=jrow[:, 2:3], in0=jrow[:, 1:2],
                                        in1=jrow[:, 2:3], op=ALU.subtract)
                # jr = rsqrt(jv + eps), jr2 = jr^2 = 1/(jv+eps)
                nc.scalar.activation(out=jrow[:, 2:3], in_=jrow[:, 2:3],
                                     func=FT.Ln, bias=eps1, scale=1.0)
                nc.scalar.activation(out=jrow[:, 3:4], in_=jrow[:, 2:3],
                                     func=FT.Exp, scale=-1.0)
                nc.scalar.activation(out=jrow[:, 2:3], in_=jrow[:, 2:3],
                                     func=FT.Exp, scale=-0.5)
                # v2 = varc * jr2 ; r2 = rsqrt(v2+eps); gg = r2*jr; h = mc*gg
                nc.vector.tensor_scalar_mul(out=varc, in0=varc,
                                            scalar1=jrow[:, 3:4])
                nc.scalar.activation(out=varc, in_=varc, func=FT.Ln,
                                     bias=eps1, scale=1.0)
                grow2 = tp.tile([1, S], f32r, name="grow2", tag="grow",
                                bufs=1)
                hrow2 = tp.tile([1, S], bf16, name="hrow2", tag="hrow",
                                bufs=1)
                nc.scalar.activation(out=grow2, in_=varc, func=FT.Exp,
                                     scale=-0.5)
                nc.vector.tensor_scalar_mul(out=grow2, in0=grow2,
                                            scalar1=jrow[:, 2:3])
                nc.vector.tensor_mul(out=hrow2, in0=mc, in1=grow2)
                z2b = tp.tile([D, S], bf16, name="z2b", tag="b2k", bufs=2)
                apply_rows(y, grow2, hrow2, "cm_ln_w", "cm_ln_b", z2b)

                # ---- pw1 + swish -> z1 (padded bf16, 2 channel chunks) ----
                z1 = []
                for oc in range(2):
                    zt = bp.tile([D, SP], bf16, name=f"z1_{oc}_{b}",
                                 tag=f"z1_{oc}")
                    nc.vector.memset(zt[:, 0:PAD], 0.0)
                    nc.vector.memset(zt[:, S + PAD:], 0.0)
                    for sb in range(2):
                        pz1 = pk.tile([D, 512], f32, tag="pk")
                        nc.tensor.matmul(pz1,
                                         s_pw1T[:, oc * D:(oc + 1) * D],
                                         z2b[:, sb * 512:(sb + 1) * 512],
                                         start=True, stop=True)
                        nc.scalar.activation(
                            out=zt[:, PAD + sb * 512:PAD + (sb + 1) * 512],
                            in_=pz1, func=FT.Silu,
                            bias=vcol(f"pw1_b{oc}"), scale=1.0)
                    z1.append(zt)

                # ---- dw conv + GLU ----
                zg = bp.tile([D, S], f32, name=f"zg{b}", tag="zg")
                for sb in range(2):
                    pa_ = pk.tile([D, 512], f32, tag="pk")
                    pg_ = pk.tile([D, 512], f32, tag="pk")
                    for oc, pt, rv_ in ((0, pa_, "dw_b0"), (1, pg_, "dw_b1")):
                        nc.tensor.matmul(pt, rrow(rv_), ones_row_b,
                                         start=True, stop=False)
                        for ic in range(2):
                            for k in range(KS):
                                nc.tensor.matmul(
                                    pt, s_dwT[:, ic, k, oc, :],
                                    z1[ic][:, sb * 512 + k:sb * 512 + k + 512],
                                    start=False,
                                    stop=(ic == 1 and k == KS - 1))
                    sg = tp.tile([D, 512], f32, name="sg", tag="sg", bufs=1)
                    nc.scalar.activation(out=sg, in_=pg_, func=FT.Sigmoid,
                                         scale=1.0)
                    nc.vector.tensor_mul(out=zg[:, sb * 512:(sb + 1) * 512],
                                         in0=pa_, in1=sg)

                # ---- BN partial stats (mean, var per channel) ----
                st = tp.tile([D, 2, 6], f32, name="bnst", tag="bnst", bufs=1)
                for c in range(2):
                    nc.vector.bn_stats(out=st[:, c, :],
                                       in_=zg[:, c * 512:(c + 1) * 512])
                mv = bp.tile([D, 2], f32, name=f"mv{b}", tag="mv")
                nc.vector.bn_aggr(out=mv, in_=st)

                x1p_f.append(x1f)
                zglu.append(zg)
                bnmv.append(mv)

            # ================= BN stats allreduce =================
            bnp = tp.tile([D, 2], f32, name="bnp", tag="bnp", bufs=1)
            # col0 = m0+m1 ; col1 = (v0+m0^2)+(v1+m1^2)
            nc.vector.tensor_add(out=bnp[:, 0:1], in0=bnmv[0][:, 0:1],
                                 in1=bnmv[1][:, 0:1])
            t0 = tp.tile([D, 2], f32, name="bnt0", tag="bnt0", bufs=1)
            nc.vector.tensor_mul(out=t0[:, 0:1], in0=bnmv[0][:, 0:1],
                                 in1=bnmv[0][:, 0:1])
            nc.vector.tensor_mul(out=t0[:, 1:2], in0=bnmv[1][:, 0:1],
                                 in1=bnmv[1][:, 0:1])
            nc.vector.tensor_add(out=t0[:, 0:1], in0=t0[:, 0:1],
                                 in1=bnmv[0][:, 1:2])
            nc.vector.tensor_add(out=t0[:, 1:2], in0=t0[:, 1:2],
                                 in1=bnmv[1][:, 1:2])
            nc.vector.tensor_add(out=bnp[:, 1:2], in0=t0[:, 0:1],
                                 in1=t0[:, 1:2])
            cin = dr.tile([D, 2], f32, name="cin", tag="cin")
            cout = dr.tile([D, 2], f32, name="cout", tag="cout")
            nc.gpsimd.dma_start(out=cin[:], in_=bnp)
            nc.gpsimd.collective_compute(
                "AllReduce", ALU.add,
                replica_groups=[list(range(NCORES))],
                ins=[cin.opt()], outs=[cout.opt()],
            )
            bns = tp.tile([D, 2], f32, name="bns", tag="bns", bufs=1)
            nc.gpsimd.dma_start(out=bns, in_=cout[:])
            # bm = c0/16 ; bE2 = c1/16 ; bv = bE2 - bm^2
            sA = tp.tile([D, 1], f32, name="sA", tag="sA", bufs=1)
            sB = tp.tile([D, 1], f32, name="sB", tag="sB", bufs=1)
            bm = tp.tile([D, 2], f32, name="bmv", tag="bmv", bufs=1)
            nc.vector.tensor_scalar_mul(out=bm[:, 0:1], in0=bns[:, 0:1],
                                        scalar1=1.0 / B)
            nc.vector.tensor_scalar_mul(out=bm[:, 1:2], in0=bns[:, 1:2],
                                        scalar1=1.0 / B)
            nc.vector.tensor_mul(out=sA, in0=bm[:, 0:1], in1=bm[:, 0:1])
            nc.vector.tensor_tensor(out=sA, in0=bm[:, 1:2], in1=sA,
                                    op=ALU.subtract)
            # br = rsqrt(bv+eps)
            nc.scalar.activation(out=sA, in_=sA, func=FT.Ln, bias=eps_col,
                                 scale=1.0)
            nc.scalar.activation(out=sA, in_=sA, func=FT.Exp, scale=-0.5)
            # sA = br*w ; sB = b - bm*sA = (bm*sA) * -1 + b
            nc.vector.tensor_scalar_mul(out=sA, in0=sA, scalar1=vcol("bn_w"))
            nc.vector.tensor_mul(out=sB, in0=bm[:, 0:1], in1=sA)
            nc.vector.tensor_scalar(out=sB, in0=sB, scalar1=-1.0,
                                    scalar2=vcol("bn_b"),
                                    op0=ALU.mult, op1=ALU.add)
            # ================= per-batch stage B =================
            for b in range(BPC):
                zbn = tp.tile([D, S], bf16, name="zbn", tag="b2k", bufs=2)
                nc.vector.tensor_scalar(out=zbn, in0=zglu[b],
                                        scalar1=sA, scalar2=sB,
                                        op0=ALU.mult, op1=ALU.add)
                x2 = bp.tile([D, S], f32, name=f"x2{b}", tag="inT")
                for sb in range(2):
                    ssl = slice(sb * 512, (sb + 1) * 512)
                    pw2p = pk.tile([D, 512], f32, tag="pk")
                    nc.tensor.matmul(pw2p, rrow("pw2_b"), ones_row_b,
                                     start=True, stop=False)
                    nc.tensor.matmul(pw2p, s_pw2T, zbn[:, ssl], start=False,
                                     stop=True)
                    nc.vector.tensor_add(
                        out=x2[:, ssl], in0=pw2p,
                        in1=x1p_f[b][:, PAD + sb * 512:PAD + (sb + 1) * 512])

                # ---- FFN ----
                growf, hrowf = part_ln_rows(x2, f"lnf_{b}")
                fbf = bp.tile([D, S], bf16, name=f"fbf{b}", tag="xbf")
                apply_rows(x2, growf, hrowf, "ff_ln_w", "ff_ln_b", fbf)
                pos = []
                for sb in range(2):
                    po = pz.tile([D, 512], f32, tag="pz")
                    nc.tensor.matmul(po, rrow("ff_b2"), ones_row_b,
                                     start=True, stop=False)
                    pos.append(po)
                for half in range(2):
                    ht = tp.tile([D, 2, S], bf16, name=f"ht{half}", tag="ht",
                                 bufs=1)
                    for i in range(2):
                        hc = half * 2 + i
                        for sb in range(2):
                            ssl = slice(sb * 512, (sb + 1) * 512)
                            ph = pk.tile([D, 512], f32, tag="pk")
                            nc.tensor.matmul(ph,
                                             s_ff1T[:, hc * D:(hc + 1) * D],
                                             fbf[:, ssl], start=True,
                                             stop=True)
                            nc.scalar.activation(out=ht[:, i, ssl], in_=ph,
                                                 func=FT.Silu,
                                                 bias=vcol(f"ff_b1_{hc}"),
                                                 scale=1.0)
                    for sb in range(2):
                        ssl = slice(sb * 512, (sb + 1) * 512)
                        for i in range(2):
                            nc.tensor.matmul(
                                pos[sb], s_ff2T[:, half * 2 + i, :],
                                ht[:, i, ssl], start=False,
                                stop=(half == 1 and i == 1))
                ofin = tp.tile([D, S], f32, name="ofin", tag="f4k", bufs=4)
                for sb in range(2):
                    ssl = slice(sb * 512, (sb + 1) * 512)
                    nc.vector.tensor_add(out=ofin[:, ssl], in0=pos[sb],
                                         in1=x2[:, ssl])
                nc.gpsimd.dma_start(out=outT[b], in_=ofin)

    nc.compile()
    return nc


def _get_nc():
    if "nc" not in _CACHE:
        _CACHE["nc"] = _build()
    return _CACHE["nc"]


def _marshal(inputs):
    I = inputs
    bf = ml_dtypes.bfloat16

    vecs = np.zeros((D, len(VC)), np.float32)
    vecs[:, VCI["ln_attn_w"]] = I["ln_attn_w"]
    vecs[:, VCI["ln_attn_b"]] = I["ln_attn_b"]
    vecs[:, VCI["qu_bias"]] = (I["bq"] + I["u_bias"].reshape(-1)) * ISQ
    vecs[:, VCI["qv_bias"]] = (I["bq"] + I["v_bias"].reshape(-1)) * ISQ
    vecs[:, VCI["bk"]] = I["bk"]
    vecs[:, VCI["dcnn_b"]] = I["dcnn_b"]
    vecs[:, VCI["cm_ln_w"]] = I["cm_ln_w"]
    vecs[:, VCI["cm_ln_b"]] = I["cm_ln_b"]
    vecs[:, VCI["bn_w"]] = I["cm_bn_w"]
    vecs[:, VCI["bn_b"]] = I["cm_bn_b"]
    vecs[:, VCI["ff_ln_w"]] = I["ff_ln_w"]
    vecs[:, VCI["ff_ln_b"]] = I["ff_ln_b"]
    vecs[:, VCI["pw1_b0"]] = I["cm_pw1_b"][0:D]
    vecs[:, VCI["pw1_b1"]] = I["cm_pw1_b"][D:2 * D]
    for c in range(4):
        vecs[:, VCI[f"ff_b1_{c}"]] = I["ff_b1"][c * D:(c + 1) * D]

    rowv = np.zeros((1, len(RV) * D), np.float32)
    rowv[0, RVI["bv"] * D:(RVI["bv"] + 1) * D] = I["bv"]
    rowv[0, RVI["bo"] * D:(RVI["bo"] + 1) * D] = I["bo"]
    rowv[0, RVI["ff_b2"] * D:(RVI["ff_b2"] + 1) * D] = I["ff_b2"]
    rowv[0, RVI["dw_b0"] * D:(RVI["dw_b0"] + 1) * D] = I["cm_dw_b"][0:D]
    rowv[0, RVI["dw_b1"] * D:(RVI["dw_b1"] + 1) * D] = I["cm_dw_b"][D:2 * D]
    rowv[0, RVI["pw2_b"] * D:(RVI["pw2_b"] + 1) * D] = I["cm_pw2_b"]

    wq = np.ascontiguousarray(I["wq"].T).astype(bf)
    wk = np.ascontiguousarray(I["wk"].T).astype(bf)
    wv = np.ascontiguousarray(I["wv"].T).astype(bf)
    wp_ = np.ascontiguousarray(I["wp"].T).astype(bf)
    wo = np.ascontiguousarray(I["wo"].T).astype(bf)
    peT = np.ascontiguousarray(_pe_host().T).astype(bf)
    dcnnT = np.ascontiguousarray(I["dcnn_w"].transpose(1, 2, 0)).astype(bf)
    pw1T = np.ascontiguousarray(I["cm_pw1_w"][:, :, 0].T).astype(bf)
    w5 = I["cm_dw_w"].reshape(2, D, 2, D, KS)
    dwT = np.ascontiguousarray(w5.transpose(3, 2, 4, 0, 1)).astype(bf)
    pw2T = np.ascontiguousarray(I["cm_pw2_w"][:, :, 0].T).astype(bf)
    ff1T = np.ascontiguousarray(I["ff_w1"].T).astype(bf)
    ff2T = np.ascontiguousarray(
        I["ff_w2"].reshape(D, EXP, D).transpose(2, 1, 0)).astype(bf)

    shared = {
        "vecs": vecs, "rowv": rowv.astype(bf),
        "wqT": wq, "wkT": wk, "wvT": wv, "wpT": wp_, "woT": wo, "peT": peT,
        "dcnnT": dcnnT, "pw1T": pw1T, "dwT": dwT, "pw2T": pw2T,
        "ff1T": ff1T, "ff2T": ff2T,
    }
    xt_all = np.ascontiguousarray(
        I["inputs"].transpose(0, 2, 1)).astype(np.float32)
    in_maps = []
    for c in range(NCORES):
        m = dict(shared)
        m["xT"] = xt_all[c * BPC:(c + 1) * BPC]
        in_maps.append(m)
    return in_maps


def kernel(**inputs):
    global LAST_EXEC_TIME_NS
    inputs = {k: np.asarray(v) for k, v in inputs.items()}
    nc = _get_nc()
    in_maps = _marshal(inputs)
    res = run_bass_kernel_spmd(nc, in_maps, core_ids=list(range(NCORES)),
                               trace=TRACE)
    if TRACE:
        global LAST_TRACE_PATH
        LAST_EXEC_TIME_NS = res.exec_time_ns
        if res.instructions_and_trace is not None:
            LAST_TRACE_PATH = res.instructions_and_trace[1]
    out = np.empty((B, S, D), np.float32)
    for c in range(NCORES):
        ot = np.asarray(res.results[c]["outT"])
        out[c * BPC:(c + 1) * BPC] = ot.transpose(0, 2, 1)
    return out
